# revision 2
# baseline (speedup 1.0000x reference)
# Trainium2 Bass kernel for batched int8-range BMM with scalar rescale:
#   out[b] = (a[b] @ b_in[b]).astype(f32) * alpha
#
# Strategy (pure batch parallelism, no communication):
#   - B=32 batches sharded 4-per-core across 8 NeuronCores.
#   - Operands hold ints in [0, 127), so a bf16 matmul with fp32 PSUM
#     accumulation is bit-exact: values <= 126 are exact in bf16, every
#     product <= 15876 and every partial sum <= 126*126*1024 < 2^24 is
#     exact in fp32. Host casts int32 -> bf16 (4x less DMA than int32).
#   - Per batch: A^T (kxm) and B (kxn) fully resident in SBUF as 8
#     [128, 1024] bf16 chunks each; 8x2 output tiles of [128, 512]
#     accumulate 8 matmuls in one PSUM bank, DVE applies the alpha scale
#     on PSUM->SBUF eviction, DMA streams f32 tiles to DRAM.
#   - Input chunks double-buffered across batches so the PE never idles.

import numpy as np
import ml_dtypes

import concourse.bass as bass
import concourse.mybir as mybir
import concourse.tile as tile
from concourse import bacc
from concourse.bass_utils import run_bass_kernel_spmd

B, M, K, N = 32, 1024, 1024, 1024
N_CORES = 8
BPC = B // N_CORES  # batches per core
P = 128
FREE = 512  # one fp32 PSUM bank


def build_kernel(alpha: float, bpc: int = BPC, m: int = M, k: int = K, n: int = N):
    nc = bacc.Bacc("TRN2", target_bir_lowering=False, debug=False)
    a_t = nc.dram_tensor("a_t", (bpc, k, m), mybir.dt.bfloat16, kind="ExternalInput")
    b_in = nc.dram_tensor("b_in", (bpc, k, n), mybir.dt.bfloat16, kind="ExternalInput")
    out = nc.dram_tensor("out", (bpc, m, n), mybir.dt.float32, kind="ExternalOutput")

    kt, mt = k // P, m // P
    free = min(FREE, n)
    nt = n // free

    with tile.TileContext(nc) as tc:
        with (
            tc.tile_pool(name="a_pool", bufs=2 * kt) as a_pool,
            tc.tile_pool(name="b_pool", bufs=2 * kt) as b_pool,
            tc.tile_pool(name="o_pool", bufs=6) as o_pool,
            tc.tile_pool(name="psum", bufs=4, space="PSUM") as psum_pool,
        ):
            for bi in range(bpc):
                a_tiles = []
                b_tiles = []
                for ko in range(kt):
                    at = a_pool.tile([P, m], mybir.dt.bfloat16, tag="a")
                    nc.sync.dma_start(at[:], a_t[bi, ko * P : (ko + 1) * P, :])
                    a_tiles.append(at)
                    bt = b_pool.tile([P, n], mybir.dt.bfloat16, tag="b")
                    nc.sync.dma_start(bt[:], b_in[bi, ko * P : (ko + 1) * P, :])
                    b_tiles.append(bt)
                for mi in range(mt):
                    for ni in range(nt):
                        ps = psum_pool.tile([P, free], mybir.dt.float32, tag="ps")
                        for ko in range(kt):
                            nc.tensor.matmul(
                                ps[:],
                                a_tiles[ko][:, mi * P : (mi + 1) * P],
                                b_tiles[ko][:, ni * free : (ni + 1) * free],
                                start=(ko == 0),
                                stop=(ko == kt - 1),
                            )
                        ot = o_pool.tile([P, free], mybir.dt.float32, tag="o")
                        nc.vector.tensor_scalar_mul(ot[:], ps[:], alpha)
                        nc.sync.dma_start(
                            out[
                                bi,
                                mi * P : (mi + 1) * P,
                                ni * free : (ni + 1) * free,
                            ],
                            ot[:],
                        )
    nc.compile()
    return nc


def prepare(a: np.ndarray, b: np.ndarray, alpha: np.ndarray):
    alpha_f = float(np.asarray(alpha).reshape(-1)[0])
    a_bf = a.astype(ml_dtypes.bfloat16)
    b_bf = np.ascontiguousarray(b.astype(ml_dtypes.bfloat16))
    a_tr = np.ascontiguousarray(a_bf.transpose(0, 2, 1))  # [B, K, M]

    nc = build_kernel(alpha_f)
    in_maps = [
        {
            "a_t": a_tr[c * BPC : (c + 1) * BPC],
            "b_in": b_bf[c * BPC : (c + 1) * BPC],
        }
        for c in range(N_CORES)
    ]
    return nc, in_maps


def kernel(a: np.ndarray, b: np.ndarray, alpha: np.ndarray) -> np.ndarray:
    nc, in_maps = prepare(a, b, alpha)
    res = run_bass_kernel_spmd(nc, in_maps, core_ids=list(range(N_CORES)))
    return np.concatenate([r["out"] for r in res.results], axis=0)
